# revision 24
# baseline (speedup 1.0000x reference)
"""Expert-parallel MoE MLP (8 experts -> 8 NeuronCores) Bass kernel for TRN2.

Problem: y[t] = W2[e] @ gelu(W1[e] @ x[t] + b1[e]) + b2[e], tokens contiguous
per expert, 2048 tokens/expert, d_in=d_out=1024, d_hid=4096.

Sharding: expert-parallel. Core e gets expert e's weights and its 2048 tokens.
No cross-core communication needed; host does the shard/unshard.

Per-core compute layout (everything partition-major, h kept as [hid, tok]):
  GEMM1: h[hid, tok]  = w1T[k,:].T @ xT[k, tok]   (accum over k = d_in tiles)
         k-tiles 0-1 run as ONE fp8e4 DoubleRow matmul (PE does 2 fp8
         MACs/cell/cycle -> 2 k-tiles in ~1.13x the time of one bf16 MM);
         k-tiles 2-7 run in fp16.  The fp8 operands are pre-scaled on host
         (x*32, w1*4096 -> product 2^17) and the fp16 w1 slice is pre-scaled
         by exactly 2^17, so both paths accumulate into the SAME PSUM bank;
         the gelu evacuation un-scales with ACT's scale=2^-17.
         Measured end-to-end max-rel error with this split: 1.62e-2
         (deterministic harness inputs; gate is 2e-2).
  GELU : h = gelu(2^-17 * psum + b1) via ScalarE with fused bias
  GEMM2: y[dout, tok] = w2T[k,:].T @ h[k, tok]    (all fp16)
  BIAS : y += b2 via ScalarE Identity with fused bias (fp16 out)

DMA architecture (hard-won): the Scalar engine's instruction FIFO must stay
clear of DMA issues (its gelu stream recycles PSUM banks); the critical
stream goes on the Sync HWDGE queue in exact consumption order; SWDGE
(gpsimd) is slow (~48 GB/s) - tiny/slack transfers only.
"""
import sys

sys.path.insert(0, "/opt/trn_rl_repo")

import numpy as np
import ml_dtypes

import concourse.bass as bass  # noqa: F401
import concourse.tile as tile
from concourse import bacc, mybir
from concourse.bass_utils import run_bass_kernel_spmd

E = 8
T_PER_E = 2048
D_IN = 1024
D_HID = 4096
D_OUT = 1024

TOK_BLK = 512          # tokens per block (= PSUM bank free size in fp32)
N_TOK_BLK = T_PER_E // TOK_BLK
K1 = D_IN // 128       # k tiles for GEMM1
KF8 = 2                # GEMM1 k-tiles 0..KF8-1 in fp8 DoubleRow
KF16 = K1 - KF8        # GEMM1 k-tiles in fp16
M1 = D_HID // 128      # output row tiles for GEMM1
K2 = D_HID // 128      # k tiles for GEMM2
M2 = D_OUT // 128      # output row tiles for GEMM2

CDT = mybir.dt.float16   # fp16 compute dtype
F8 = mybir.dt.float8e4   # TRN e4m3 (max 240)
NP_CDT = np.float16
NP_F8 = ml_dtypes.float8_e4m3

S_X8 = 32.0              # fp8 x scale
S_W18 = 4096.0           # fp8 w1 scale
W1_SCALE = S_X8 * S_W18  # 2^17: folded into the fp16 w1 slice (exact in fp16)
ACT1_SCALE = 1.0 / W1_SCALE

N_WARM = 14              # warmup matmuls: bridge prologue end -> first data

XW16 = KF16 * TOK_BLK    # 3072 fp16 x cols per token block
XW8 = KF8 * TOK_BLK      # 1024 fp8 x cols per token block
MW16 = KF16 * 128        # 768 fp16 w1 cols per m-tile
MW8 = KF8 * 128          # 256 fp8 w1 cols per m-tile

_compiled = None


def _build():
    nc = bacc.Bacc("TRN2", target_bir_lowering=False, debug=False)

    # Host-permuted layouts (see _make_in_maps):
    #   x8L [128, t*1024 + j*512 + c]  = 32 * x[t*512+c, j*128+p]      (fp8)
    #   xL  [128, t*3072 + k'*512 + c] = x[t*512+c, (k'+2)*128+p]      (fp16)
    #   w18L[128, m*256 + j*128 + mc]  = 4096 * w1[m*128+mc, j*128+p]  (fp8)
    #   w1L [128, m*768 + k'*128 + mc] = 2^17 * w1[m*128+mc, (k'+2)*128+p]
    #   w2L [128, d*4096 + k*128 + dc] = w2[d*128+dc, k*128+p]
    x8L = nc.dram_tensor("x8L", [128, N_TOK_BLK * XW8], F8, kind="ExternalInput").ap()
    xL = nc.dram_tensor("xL", [128, N_TOK_BLK * XW16], CDT, kind="ExternalInput").ap()
    w18L = nc.dram_tensor("w18L", [128, M1 * MW8], F8, kind="ExternalInput").ap()
    w1L = nc.dram_tensor("w1L", [128, M1 * MW16], CDT, kind="ExternalInput").ap()
    w2L = nc.dram_tensor("w2L", [128, M2 * K2 * 128], CDT, kind="ExternalInput").ap()
    b1r = nc.dram_tensor("b1r", [128, M1], mybir.dt.float32, kind="ExternalInput").ap()
    b2r = nc.dram_tensor("b2r", [128, M2], mybir.dt.float32, kind="ExternalInput").ap()
    yT = nc.dram_tensor("yT", [D_OUT, T_PER_E], CDT, kind="ExternalOutput").ap()

    DR = mybir.MatmulPerfMode.DoubleRowSwInterleave

    with tile.TileContext(nc) as tc:
        with tc.tile_pool(name="wpool", bufs=1) as wpool, \
             tc.tile_pool(name="xpool", bufs=2) as xpool, \
             tc.tile_pool(name="x8pool", bufs=2) as x8pool, \
             tc.tile_pool(name="hpool", bufs=1) as hpool, \
             tc.tile_pool(name="opool", bufs=4) as opool, \
             tc.tile_pool(name="ps1", bufs=4, space="PSUM") as ps1, \
             tc.tile_pool(name="ps2", bufs=4, space="PSUM") as ps2:

            # --- PE warmup scratch (memset on GpSimd: earliest idle engine)
            scr = wpool.tile([128, 128], CDT, name="scr")
            nc.gpsimd.memset(scr[:], 0.0)
            jnk = wpool.tile([128, 1], mybir.dt.float32, name="jnk")
            nc.gpsimd.memset(jnk[:], 0.0)

            for i in range(N_WARM):
                wps = ps1.tile([128, 128], mybir.dt.float32, tag="ps1", name=f"warm{i}")
                nc.tensor.matmul(wps[:], scr[:], scr[:], start=True, stop=True)

            # Scalar: b1, then the gelu-table preload
            b1_sb = wpool.tile([128, M1], mybir.dt.float32, name="b1_sb")
            nc.scalar.dma_start(b1_sb[:], b1r[:, :])
            jnk2 = wpool.tile([128, 1], mybir.dt.float32, name="jnk2")
            nc.scalar.activation(jnk2[:], jnk[:],
                                 mybir.ActivationFunctionType.Gelu,
                                 scale=1.0)

            # === Sync queue, strict consumption order ===
            x_blocks = {}
            x8_blocks = {}
            x_sb = xpool.tile([128, XW16], CDT, tag="x", name="x_sb0")
            x8_sb = x8pool.tile([128, XW8], F8, tag="x8", name="x8_sb0")
            x_blocks[0] = x_sb
            x8_blocks[0] = x8_sb
            w1_sb = wpool.tile([128, M1 * MW16], CDT, name="w1_sb")
            w18_sb = wpool.tile([128, M1 * MW8], F8, name="w18_sb")
            w2_sb = wpool.tile([128, M2 * K2 * 128], CDT, name="w2_sb")
            dw = K2 * 128

            # first DoubleRow MM needs only x8 b0 (128KB) + w18 m0 (32KB)
            nc.sync.dma_start(x8_sb[:], x8L[:, 0:XW8])
            nc.sync.dma_start(w18_sb[:, 0:MW8], w18L[:, 0:MW8])
            # fp16 k-pieces of block 0 + w1 m0, then the m-tile streams
            nc.sync.dma_start(x_sb[:, 0:TOK_BLK], xL[:, 0:TOK_BLK])
            nc.sync.dma_start(w1_sb[:, 0:MW16 // 2], w1L[:, 0:MW16 // 2])
            nc.sync.dma_start(w1_sb[:, MW16 // 2:MW16], w1L[:, MW16 // 2:MW16])
            nc.sync.dma_start(x_sb[:, TOK_BLK:3 * TOK_BLK], xL[:, TOK_BLK:3 * TOK_BLK])
            nc.sync.dma_start(x_sb[:, 3 * TOK_BLK:XW16], xL[:, 3 * TOK_BLK:XW16])
            for m in range(1, M1):
                if m % 8 == 1:  # w18 for the next 8 m-tiles rides ahead
                    lo, hi = m * MW8, min(M1, m + 8) * MW8
                    nc.sync.dma_start(w18_sb[:, lo:hi], w18L[:, lo:hi])
                nc.sync.dma_start(w1_sb[:, m * MW16:(m + 1) * MW16],
                                  w1L[:, m * MW16:(m + 1) * MW16])
            for d in range(M2):
                nc.sync.dma_start(w2_sb[:, d * dw:(d + 1) * dw],
                                  w2L[:, d * dw:(d + 1) * dw])

            # SWDGE: non-critical b2 + x block 1
            b2_sb = wpool.tile([128, M2], mybir.dt.float32, name="b2_sb")
            nc.gpsimd.dma_start(b2_sb[:], b2r[:, :])
            x8_sb1 = x8pool.tile([128, XW8], F8, tag="x8", name="x8_sb1")
            nc.gpsimd.dma_start(x8_sb1[:], x8L[:, XW8:2 * XW8])
            x8_blocks[1] = x8_sb1
            x_sb1 = xpool.tile([128, XW16], CDT, tag="x", name="x_sb1")
            nc.gpsimd.dma_start(x_sb1[:], xL[:, XW16:2 * XW16])
            x_blocks[1] = x_sb1

            for t in range(N_TOK_BLK):
                if t in x_blocks:
                    x_sb = x_blocks[t]
                    x8_sb = x8_blocks[t]
                else:
                    x8_sb = x8pool.tile([128, XW8], F8, tag="x8", name=f"x8_sb{t}")
                    nc.sync.dma_start(x8_sb[:], x8L[:, t * XW8:(t + 1) * XW8])
                    x_sb = xpool.tile([128, XW16], CDT, tag="x", name=f"x_sb{t}")
                    nc.sync.dma_start(x_sb[:], xL[:, t * XW16:(t + 1) * XW16])

                x8_ap = x8_sb[:, :].rearrange("p (two n) -> p two n", two=2)

                # --- GEMM1 + gelu: h[m] tiles ---
                # software pipeline depth 4 (= ps1 banks): the DoubleRow MM
                # of m-tile m+4 is emitted right after gelu(m) frees its
                # bank, so during block 0's head the 4 leading DRs (needing
                # only the small x8+w18 transfers) keep the PE busy while
                # the fp16 x/w1 stream is still landing.
                def g1_dr(m):
                    psum = ps1.tile([128, TOK_BLK], mybir.dt.float32,
                                    tag="ps1", name=f"ps1_{t}_{m}")
                    nc.tensor.matmul(
                        psum[:],
                        w18_sb[:, m * MW8:(m + 1) * MW8]
                        .rearrange("p (two m) -> p two m", two=2),
                        x8_ap,
                        start=True, stop=False, perf_mode=DR,
                    )
                    return psum

                h_tiles = []
                dr_psums = {m: g1_dr(m) for m in range(4)}
                for m in range(M1):
                    psum = dr_psums.pop(m)
                    # k2-7 in fp16 (w pre-scaled by 2^17)
                    for k in range(KF16):
                        nc.tensor.matmul(
                            psum[:],
                            w1_sb[:, m * MW16 + k * 128: m * MW16 + (k + 1) * 128],
                            x_sb[:, k * TOK_BLK:(k + 1) * TOK_BLK],
                            start=False, stop=(k == KF16 - 1),
                        )
                    h_sb = hpool.tile([128, TOK_BLK], CDT, tag=f"h{m}",
                                      name=f"h_sb{t}_{m}")
                    nc.scalar.activation(h_sb[:], psum[:],
                                         mybir.ActivationFunctionType.Gelu,
                                         bias=b1_sb[:, m:m + 1], scale=ACT1_SCALE)
                    h_tiles.append(h_sb)
                    if m + 4 < M1:
                        dr_psums[m + 4] = g1_dr(m + 4)

                # --- GEMM2 + bias: y[d] tiles (fp16 out) ---
                for d in range(M2):
                    psum = ps2.tile([128, TOK_BLK], mybir.dt.float32,
                                    tag="ps2", name=f"ps2_{t}_{d}")
                    for k in range(K2):
                        nc.tensor.matmul(
                            psum[:],
                            w2_sb[:, d * (K2 * 128) + k * 128: d * (K2 * 128) + (k + 1) * 128],
                            h_tiles[k][:],
                            start=(k == 0), stop=(k == K2 - 1),
                        )
                    o_sb = opool.tile([128, TOK_BLK], CDT,
                                      tag="o", name=f"o_sb{t}_{d}")
                    nc.scalar.activation(o_sb[:], psum[:],
                                         mybir.ActivationFunctionType.Identity,
                                         bias=b2_sb[:, d:d + 1], scale=1.0)
                    nc.sync.dma_start(yT[d * 128:(d + 1) * 128,
                                         t * TOK_BLK:(t + 1) * TOK_BLK],
                                      o_sb[:])

    nc.compile()
    return nc


def _get_compiled():
    global _compiled
    if _compiled is None:
        _compiled = _build()
    return _compiled


def _to_f8(v):
    return np.clip(v, -240.0, 240.0).astype(NP_F8)


def _make_in_maps(x, w1, b1, w2, b2):
    in_maps = []
    for e in range(E):
        xe = x[e * T_PER_E:(e + 1) * T_PER_E]            # [2048, 1024]
        # fp8 slice: dims 0..255 (k-tiles 0-1)
        x8 = xe[:, :KF8 * 128].reshape(N_TOK_BLK, TOK_BLK, KF8, 128)
        x8 = x8.transpose(3, 0, 2, 1).reshape(128, -1)   # p, (t j c)
        # fp16 slice: dims 256..1023 (k-tiles 2-7)
        xl = xe[:, KF8 * 128:].reshape(N_TOK_BLK, TOK_BLK, KF16, 128)
        xl = xl.transpose(3, 0, 2, 1).reshape(128, -1)   # p, (t k' c)
        w1e = w1[e]                                      # [4096, 1024]
        w18 = w1e[:, :KF8 * 128].reshape(M1, 128, KF8, 128)  # m, mc, j, p
        # SwInterleave storage: col = m*256 + (127-mc)*2 + j
        w18 = w18[:, ::-1, :, :].transpose(3, 0, 1, 2).reshape(128, -1)
        w1l = w1e[:, KF8 * 128:].reshape(M1, 128, KF16, 128)
        w1l = w1l.transpose(3, 0, 2, 1).reshape(128, -1)  # p, (m k' mc)
        w2e = w2[e].reshape(M2, 128, K2, 128)
        w2l = w2e.transpose(3, 0, 2, 1).reshape(128, -1)  # p, (d k dc)
        in_maps.append({
            "x8L": _to_f8(np.ascontiguousarray(x8) * S_X8),
            "xL": np.ascontiguousarray(xl).astype(NP_CDT),
            "w18L": _to_f8(np.ascontiguousarray(w18) * S_W18),
            "w1L": (np.ascontiguousarray(w1l) * W1_SCALE).astype(NP_CDT),
            "w2L": np.ascontiguousarray(w2l).astype(NP_CDT),
            "b1r": np.ascontiguousarray(b1[e].reshape(M1, 128).T).astype(np.float32),
            "b2r": np.ascontiguousarray(b2[e].reshape(M2, 128).T).astype(np.float32),
        })
    return in_maps


def run(x, cnt, w1, b1, w2, b2, trace=False):
    nc = _get_compiled()
    in_maps = _make_in_maps(x, w1, b1, w2, b2)
    res = run_bass_kernel_spmd(nc, in_maps, core_ids=list(range(E)), trace=trace)
    outs = [res.results[e]["yT"].T for e in range(E)]
    y = np.concatenate(outs, axis=0).astype(np.float32)
    return y, res


def kernel(x, cnt, w1, b1, w2, b2):
    y, _ = run(x, cnt, w1, b1, w2, b2, trace=False)
    return y


# revision 25
# speedup vs baseline: 1.0238x; 1.0238x over previous
"""Expert-parallel MoE MLP (8 experts -> 8 NeuronCores) Bass kernel for TRN2.

Problem: y[t] = W2[e] @ gelu(W1[e] @ x[t] + b1[e]) + b2[e], tokens contiguous
per expert, 2048 tokens/expert, d_in=d_out=1024, d_hid=4096.

Sharding: expert-parallel. Core e gets expert e's weights and its 2048 tokens.
No cross-core communication needed; host does the shard/unshard.

Per-core compute layout (everything partition-major, h kept as [hid, tok]):
  GEMM1: h[hid, tok]  = w1T[k,:].T @ xT[k, tok]   (accum over k = d_in tiles)
         k-tiles 0-1 run as ONE fp8e4 DoubleRow matmul (PE does 2 fp8
         MACs/cell/cycle -> 2 k-tiles in ~1.13x the time of one bf16 MM);
         k-tiles 2-7 run in fp16.  The fp8 operands are pre-scaled on host
         (x*32, w1*4096 -> product 2^17) and the fp16 w1 slice is pre-scaled
         by exactly 2^17, so both paths accumulate into the SAME PSUM bank;
         the gelu evacuation un-scales with ACT's scale=2^-17.
         Measured end-to-end max-rel error with this split: 1.62e-2
         (deterministic harness inputs; gate is 2e-2).
  GELU : h = gelu(2^-17 * psum + b1) via ScalarE with fused bias
  GEMM2: y[dout, tok] = w2T[k,:].T @ h[k, tok]    (all fp16)
  BIAS : y += b2 via ScalarE Identity with fused bias (fp16 out)

DMA architecture (hard-won): the Scalar engine's instruction FIFO must stay
clear of DMA issues (its gelu stream recycles PSUM banks); the critical
stream goes on the Sync HWDGE queue in exact consumption order; SWDGE
(gpsimd) is slow (~48 GB/s) - tiny/slack transfers only.
"""
import sys

sys.path.insert(0, "/opt/trn_rl_repo")

import numpy as np
import ml_dtypes

import concourse.bass as bass  # noqa: F401
import concourse.tile as tile
from concourse import bacc, mybir
from concourse.bass_utils import run_bass_kernel_spmd

E = 8
T_PER_E = 2048
D_IN = 1024
D_HID = 4096
D_OUT = 1024

TOK_BLK = 512          # tokens per block (= PSUM bank free size in fp32)
N_TOK_BLK = T_PER_E // TOK_BLK
K1 = D_IN // 128       # k tiles for GEMM1
KF8 = 2                # GEMM1 k-tiles 0..KF8-1 in fp8 DoubleRow
KF16 = K1 - KF8        # GEMM1 k-tiles in fp16
M1 = D_HID // 128      # output row tiles for GEMM1
K2 = D_HID // 128      # k tiles for GEMM2
M2 = D_OUT // 128      # output row tiles for GEMM2

CDT = mybir.dt.float16   # fp16 compute dtype
F8 = mybir.dt.float8e4   # TRN e4m3 (max 240)
NP_CDT = np.float16
NP_F8 = ml_dtypes.float8_e4m3

S_X8 = 32.0              # fp8 x scale
S_W18 = 4096.0           # fp8 w1 scale
W1_SCALE = S_X8 * S_W18  # 2^17: folded into the fp16 w1 slice (exact in fp16)
ACT1_SCALE = 1.0 / W1_SCALE

N_WARM = 14              # warmup matmuls: bridge prologue end -> first data

XW16 = KF16 * TOK_BLK    # 3072 fp16 x cols per token block
XW8 = KF8 * TOK_BLK      # 1024 fp8 x cols per token block
MW16 = KF16 * 128        # 768 fp16 w1 cols per m-tile
MW8 = KF8 * 128          # 256 fp8 w1 cols per m-tile

_compiled = None


def _build():
    nc = bacc.Bacc("TRN2", target_bir_lowering=False, debug=False)

    # Host-permuted layouts (see _make_in_maps):
    #   x8L [128, t*1024 + j*512 + c]  = 32 * x[t*512+c, j*128+p]      (fp8)
    #   xL  [128, t*3072 + k'*512 + c] = x[t*512+c, (k'+2)*128+p]      (fp16)
    #   w18L[128, m*256 + j*128 + mc]  = 4096 * w1[m*128+mc, j*128+p]  (fp8)
    #   w1L [128, m*768 + k'*128 + mc] = 2^17 * w1[m*128+mc, (k'+2)*128+p]
    #   w2L [128, d*4096 + k*128 + dc] = w2[d*128+dc, k*128+p]
    x8L = nc.dram_tensor("x8L", [128, N_TOK_BLK * XW8], F8, kind="ExternalInput").ap()
    xL = nc.dram_tensor("xL", [128, N_TOK_BLK * XW16], CDT, kind="ExternalInput").ap()
    w18L = nc.dram_tensor("w18L", [128, M1 * MW8], F8, kind="ExternalInput").ap()
    w1L = nc.dram_tensor("w1L", [128, M1 * MW16], CDT, kind="ExternalInput").ap()
    w2L = nc.dram_tensor("w2L", [128, M2 * K2 * 128], CDT, kind="ExternalInput").ap()
    b1r = nc.dram_tensor("b1r", [128, M1], mybir.dt.float32, kind="ExternalInput").ap()
    b2r = nc.dram_tensor("b2r", [128, M2], mybir.dt.float32, kind="ExternalInput").ap()
    yT = nc.dram_tensor("yT", [D_OUT, T_PER_E], CDT, kind="ExternalOutput").ap()

    DR = mybir.MatmulPerfMode.DoubleRowSwInterleave

    with tile.TileContext(nc) as tc:
        with tc.tile_pool(name="wpool", bufs=1) as wpool, \
             tc.tile_pool(name="xpool", bufs=2) as xpool, \
             tc.tile_pool(name="x8pool", bufs=2) as x8pool, \
             tc.tile_pool(name="hpool", bufs=1) as hpool, \
             tc.tile_pool(name="opool", bufs=4) as opool, \
             tc.tile_pool(name="ps1", bufs=4, space="PSUM") as ps1, \
             tc.tile_pool(name="ps2", bufs=4, space="PSUM") as ps2:

            # --- PE warmup scratch (memset on GpSimd: earliest idle engine)
            scr = wpool.tile([128, 128], CDT, name="scr")
            nc.gpsimd.memset(scr[:], 0.0)
            jnk = wpool.tile([128, 1], mybir.dt.float32, name="jnk")
            nc.gpsimd.memset(jnk[:], 0.0)

            for i in range(N_WARM):
                wps = ps1.tile([128, 128], mybir.dt.float32, tag="ps1", name=f"warm{i}")
                nc.tensor.matmul(wps[:], scr[:], scr[:], start=True, stop=True)

            # Scalar: b1, then the gelu-table preload
            b1_sb = wpool.tile([128, M1], mybir.dt.float32, name="b1_sb")
            nc.scalar.dma_start(b1_sb[:], b1r[:, :])
            jnk2 = wpool.tile([128, 1], mybir.dt.float32, name="jnk2")
            nc.scalar.activation(jnk2[:], jnk[:],
                                 mybir.ActivationFunctionType.Gelu,
                                 scale=1.0)

            # === Sync queue, strict consumption order ===
            x_blocks = {}
            x8_blocks = {}
            x_sb = xpool.tile([128, XW16], CDT, tag="x", name="x_sb0")
            x8_sb = x8pool.tile([128, XW8], F8, tag="x8", name="x8_sb0")
            x_blocks[0] = x_sb
            x8_blocks[0] = x8_sb
            w1_sb = wpool.tile([128, M1 * MW16], CDT, name="w1_sb")
            w18_sb = wpool.tile([128, M1 * MW8], F8, name="w18_sb")
            w2_sb = wpool.tile([128, M2 * K2 * 128], CDT, name="w2_sb")
            dw = K2 * 128

            # first DoubleRow MM needs only x8 b0 (128KB) + w18 m0 (32KB)
            nc.sync.dma_start(x8_sb[:], x8L[:, 0:XW8])
            nc.sync.dma_start(w18_sb[:, 0:MW8], w18L[:, 0:MW8])
            # fp16 k-pieces of block 0 + w1 m0, then the m-tile streams
            nc.sync.dma_start(x_sb[:, 0:TOK_BLK], xL[:, 0:TOK_BLK])
            nc.sync.dma_start(w1_sb[:, 0:MW16 // 2], w1L[:, 0:MW16 // 2])
            nc.sync.dma_start(w1_sb[:, MW16 // 2:MW16], w1L[:, MW16 // 2:MW16])
            nc.sync.dma_start(x_sb[:, TOK_BLK:3 * TOK_BLK], xL[:, TOK_BLK:3 * TOK_BLK])
            nc.sync.dma_start(x_sb[:, 3 * TOK_BLK:XW16], xL[:, 3 * TOK_BLK:XW16])
            for m in range(1, M1):
                if m % 8 == 1:  # w18 for the next 8 m-tiles rides ahead
                    lo, hi = m * MW8, min(M1, m + 8) * MW8
                    nc.sync.dma_start(w18_sb[:, lo:hi], w18L[:, lo:hi])
                nc.sync.dma_start(w1_sb[:, m * MW16:(m + 1) * MW16],
                                  w1L[:, m * MW16:(m + 1) * MW16])
            for d in range(M2):
                nc.sync.dma_start(w2_sb[:, d * dw:(d + 1) * dw],
                                  w2L[:, d * dw:(d + 1) * dw])

            # SWDGE: non-critical b2 + x block 1
            b2_sb = wpool.tile([128, M2], mybir.dt.float32, name="b2_sb")
            nc.gpsimd.dma_start(b2_sb[:], b2r[:, :])
            x8_sb1 = x8pool.tile([128, XW8], F8, tag="x8", name="x8_sb1")
            nc.gpsimd.dma_start(x8_sb1[:], x8L[:, XW8:2 * XW8])
            x8_blocks[1] = x8_sb1
            x_sb1 = xpool.tile([128, XW16], CDT, tag="x", name="x_sb1")
            nc.gpsimd.dma_start(x_sb1[:], xL[:, XW16:2 * XW16])
            x_blocks[1] = x_sb1

            for t in range(N_TOK_BLK):
                if t in x_blocks:
                    x_sb = x_blocks[t]
                    x8_sb = x8_blocks[t]
                else:
                    x8_sb = x8pool.tile([128, XW8], F8, tag="x8", name=f"x8_sb{t}")
                    nc.sync.dma_start(x8_sb[:], x8L[:, t * XW8:(t + 1) * XW8])
                    x_sb = xpool.tile([128, XW16], CDT, tag="x", name=f"x_sb{t}")
                    nc.sync.dma_start(x_sb[:], xL[:, t * XW16:(t + 1) * XW16])

                x8_ap = x8_sb[:, :].rearrange("p (two n) -> p two n", two=2)

                # --- GEMM1 + gelu: h[m] tiles ---
                h_tiles = []
                for m in range(M1):
                    psum = ps1.tile([128, TOK_BLK], mybir.dt.float32,
                                    tag="ps1", name=f"ps1_{t}_{m}")
                    # k0-1 as one fp8 DoubleRow matmul (scaled by 2^17)
                    nc.tensor.matmul(
                        psum[:],
                        w18_sb[:, m * MW8:(m + 1) * MW8]
                        .rearrange("p (two m) -> p two m", two=2),
                        x8_ap,
                        start=True, stop=False, perf_mode=DR,
                    )
                    # k2-7 in fp16 (w pre-scaled by 2^17)
                    for k in range(KF16):
                        nc.tensor.matmul(
                            psum[:],
                            w1_sb[:, m * MW16 + k * 128: m * MW16 + (k + 1) * 128],
                            x_sb[:, k * TOK_BLK:(k + 1) * TOK_BLK],
                            start=False, stop=(k == KF16 - 1),
                        )
                    h_sb = hpool.tile([128, TOK_BLK], CDT, tag=f"h{m}",
                                      name=f"h_sb{t}_{m}")
                    nc.scalar.activation(h_sb[:], psum[:],
                                         mybir.ActivationFunctionType.Gelu,
                                         bias=b1_sb[:, m:m + 1], scale=ACT1_SCALE)
                    h_tiles.append(h_sb)

                # --- GEMM2 + bias: y[d] tiles (fp16 out) ---
                for d in range(M2):
                    psum = ps2.tile([128, TOK_BLK], mybir.dt.float32,
                                    tag="ps2", name=f"ps2_{t}_{d}")
                    for k in range(K2):
                        nc.tensor.matmul(
                            psum[:],
                            w2_sb[:, d * (K2 * 128) + k * 128: d * (K2 * 128) + (k + 1) * 128],
                            h_tiles[k][:],
                            start=(k == 0), stop=(k == K2 - 1),
                        )
                    o_sb = opool.tile([128, TOK_BLK], CDT,
                                      tag="o", name=f"o_sb{t}_{d}")
                    nc.scalar.activation(o_sb[:], psum[:],
                                         mybir.ActivationFunctionType.Identity,
                                         bias=b2_sb[:, d:d + 1], scale=1.0)
                    nc.sync.dma_start(yT[d * 128:(d + 1) * 128,
                                         t * TOK_BLK:(t + 1) * TOK_BLK],
                                      o_sb[:])

    nc.compile()
    return nc


def _get_compiled():
    global _compiled
    if _compiled is None:
        _compiled = _build()
    return _compiled


def _to_f8(v):
    return np.clip(v, -240.0, 240.0).astype(NP_F8)


def _make_in_maps(x, w1, b1, w2, b2):
    in_maps = []
    for e in range(E):
        xe = x[e * T_PER_E:(e + 1) * T_PER_E]            # [2048, 1024]
        # fp8 slice: dims 0..255 (k-tiles 0-1)
        x8 = xe[:, :KF8 * 128].reshape(N_TOK_BLK, TOK_BLK, KF8, 128)
        x8 = x8.transpose(3, 0, 2, 1).reshape(128, -1)   # p, (t j c)
        # fp16 slice: dims 256..1023 (k-tiles 2-7)
        xl = xe[:, KF8 * 128:].reshape(N_TOK_BLK, TOK_BLK, KF16, 128)
        xl = xl.transpose(3, 0, 2, 1).reshape(128, -1)   # p, (t k' c)
        w1e = w1[e]                                      # [4096, 1024]
        w18 = w1e[:, :KF8 * 128].reshape(M1, 128, KF8, 128)  # m, mc, j, p
        # SwInterleave storage: col = m*256 + (127-mc)*2 + j
        w18 = w18[:, ::-1, :, :].transpose(3, 0, 1, 2).reshape(128, -1)
        w1l = w1e[:, KF8 * 128:].reshape(M1, 128, KF16, 128)
        w1l = w1l.transpose(3, 0, 2, 1).reshape(128, -1)  # p, (m k' mc)
        w2e = w2[e].reshape(M2, 128, K2, 128)
        w2l = w2e.transpose(3, 0, 2, 1).reshape(128, -1)  # p, (d k dc)
        in_maps.append({
            "x8L": _to_f8(np.ascontiguousarray(x8) * S_X8),
            "xL": np.ascontiguousarray(xl).astype(NP_CDT),
            "w18L": _to_f8(np.ascontiguousarray(w18) * S_W18),
            "w1L": (np.ascontiguousarray(w1l) * W1_SCALE).astype(NP_CDT),
            "w2L": np.ascontiguousarray(w2l).astype(NP_CDT),
            "b1r": np.ascontiguousarray(b1[e].reshape(M1, 128).T).astype(np.float32),
            "b2r": np.ascontiguousarray(b2[e].reshape(M2, 128).T).astype(np.float32),
        })
    return in_maps


def run(x, cnt, w1, b1, w2, b2, trace=False):
    nc = _get_compiled()
    in_maps = _make_in_maps(x, w1, b1, w2, b2)
    res = run_bass_kernel_spmd(nc, in_maps, core_ids=list(range(E)), trace=trace)
    outs = [res.results[e]["yT"].T for e in range(E)]
    y = np.concatenate(outs, axis=0).astype(np.float32)
    return y, res


def kernel(x, cnt, w1, b1, w2, b2):
    y, _ = run(x, cnt, w1, b1, w2, b2, trace=False)
    return y
